# revision 5
# baseline (speedup 1.0000x reference)
"""CKA vq_codebook kernel for 8 Trainium2 NeuronCores.

Math (verified against the reference in fp64):
  Gx[b]  = x[b] @ x[b]^T, Cxc = double-center(Gx) = (P x[b]) (P x[b])^T
  Gy[m]  = cent[m] @ cent[m]^T           (uncentered)
  hsic[b,m] = <Cxc[b], center(Gy[m])> = <Cxc[b], Gy[m]>   (centering is a
              self-adjoint projection and Cxc is already centered)
  vx[b] = ||Cxc[b]||_F
  vy[m] = ||center(Gy[m])||_F = sqrt(Q - (2/L)*sum_i R_i^2 + (S/L)^2)
          with Q = ||Gy||_F^2, R = row sums of Gy, S = total sum
  The second CKA pass is a pure gather: Cs = Cy[idx]  =>  hsic2 = hsic[:, idx],
  vs = vy[idx].

Sharding: codebook M=512 is split 64 per core (expert-style); the B=32 input
grams are computed on every core (their cost is small).  Each core computes a
96x96 Gram matrix BG of the flattened 128x128 matrices
[Cxc_0..Cxc_31, Gy_0..Gy_63]: the (b, 32+m) block is hsic, diag gives vx^2 and
Q.  Host gathers the 8 slices, does the O(B*M) normalization / log / argmax
tail, and the trivial gathered second pass.

Layout trick: the PE contracts over partitions only, so the gram G = A A^T
needs A^T (h-major) tiles in SBUF.  The host pre-transposes x and the
centroids into the exact SBUF images so the device never transposes anything.
"""

import numpy as np

B, L, H, M = 32, 128, 512, 512
N_CORES = 8
MLOC = M // N_CORES          # 64 centroids per core
G = H // 128                 # 4 contraction chunks of 128
NV = B + MLOC                # 96 vectors in the per-core big Gram
EPS = 1e-8

_BUILT = {}


def _build_bass():
    """Build the single-core Bass/Tile program (same program on all 8 cores)."""
    from concourse import bacc, mybir
    from concourse.tile import TileContext

    f32 = mybir.dt.float32
    nc = bacc.Bacc("TRN2", target_bir_lowering=False, debug=False,
                   num_devices=N_CORES)

    xt = nc.dram_tensor("xt", [B, 128, G * 128], f32, kind="ExternalInput")
    ct = nc.dram_tensor("ct", [MLOC, 128, G * 128], f32, kind="ExternalInput")
    bg_out = nc.dram_tensor("bg", [NV, NV], f32, kind="ExternalOutput")
    rt_out = nc.dram_tensor("rt", [128, MLOC], f32, kind="ExternalOutput")

    with TileContext(nc) as tc:
        with (
            tc.tile_pool(name="big", bufs=1) as bigp,
            tc.tile_pool(name="stage", bufs=8) as stage,
            tc.tile_pool(name="psg", bufs=4, space="PSUM") as psg,
            tc.tile_pool(name="psbg", bufs=1, space="PSUM") as psbg,
            tc.tile_pool(name="outp", bufs=1) as outp,
        ):
            # v-major: block v occupies free columns [128v, 128v+128)
            big = bigp.tile([128, NV * 128], f32)
            rt = outp.tile([128, MLOC], f32)

            def gram_block(src_dram, v, with_rowsum_col=None):
                st = stage.tile([128, G * 128], f32, tag="stage")
                nc.gpsimd.dma_start(st[:], src_dram)
                ps = psg.tile([128, 128], f32, tag="gram")
                for g in range(G):
                    sl = st[:, 128 * g:128 * (g + 1)]
                    nc.tensor.matmul(ps[:], sl, sl,
                                     start=(g == 0), stop=(g == G - 1))
                dst = big[:, 128 * v:128 * (v + 1)]
                if v % 2 == 0:
                    nc.scalar.activation(dst, ps[:],
                                         mybir.ActivationFunctionType.Copy)
                else:
                    nc.vector.tensor_copy(dst, ps[:])
                if with_rowsum_col is not None:
                    nc.vector.tensor_reduce(
                        rt[:, with_rowsum_col:with_rowsum_col + 1], ps[:],
                        axis=mybir.AxisListType.X, op=mybir.AluOpType.add)

            for b in range(B):
                gram_block(xt[b], b)
            for m in range(MLOC):
                gram_block(ct[m], B + m, with_rowsum_col=m)

            # Big Gram over the 96 flattened (128x128) tiles, contracting the
            # 16384-long flat index as 128 chunks of 128 partitions.
            bg_ps = psbg.tile([NV, NV], f32)
            big3 = big[:].rearrange("p (v i) -> p v i", i=128)
            for i in range(128):
                sl = big3[:, :, i]            # (128, 96), free stride 128
                nc.tensor.matmul(bg_ps[:], sl, sl,
                                 start=(i == 0), stop=(i == 127))
            bg_sb = outp.tile([NV, NV], f32)
            nc.vector.tensor_copy(bg_sb[:], bg_ps[:])
            nc.sync.dma_start(bg_out[:], bg_sb[:])
            nc.sync.dma_start(rt_out[:], rt[:])

    nc.compile()
    return nc


def _get_nc():
    if "nc" not in _BUILT:
        _BUILT["nc"] = _build_bass()
    return _BUILT["nc"]


def _sbuf_image(a):
    """(N, L, H) row-major -> (N, 128, 512) SBUF image with
    img[n, p, g*128 + l] = a[n, l, 128*g + p]."""
    n = a.shape[0]
    return np.ascontiguousarray(
        a.reshape(n, L, G, 128).transpose(0, 3, 2, 1).reshape(n, 128, G * 128))


def _prepare_inputs(x, centroid_w):
    x64 = np.asarray(x, np.float64)
    xc = (x64 - x64.mean(axis=1, keepdims=True)).astype(np.float32)
    xt_img = _sbuf_image(xc)
    cent = np.asarray(centroid_w, np.float32).reshape(M, L, H)
    in_maps = []
    for c in range(N_CORES):
        ct_img = _sbuf_image(cent[c * MLOC:(c + 1) * MLOC])
        in_maps.append({"xt": xt_img, "ct": ct_img})
    return in_maps


def _postprocess(results):
    hsic = np.hstack([r["bg"][0:B, B:NV] for r in results]).astype(np.float64)
    diag0 = np.diagonal(results[0]["bg"]).astype(np.float64)
    vx = np.sqrt(diag0[0:B])
    Q = np.concatenate([np.diagonal(r["bg"])[B:NV] for r in results]).astype(
        np.float64)
    R = np.hstack([r["rt"] for r in results]).astype(np.float64)  # (128, M)
    S = R.sum(axis=0)
    sumR2 = (R * R).sum(axis=0)
    vy = np.sqrt(Q - (2.0 / L) * sumR2 + (S / L) ** 2)

    s = np.abs(hsic) / (vx[:, None] * vy[None, :])
    mat = (-np.log(s + EPS)).astype(np.float32)
    idx = np.argmax(mat, axis=1).astype(np.int32)

    hsic2 = hsic[:, idx]
    vs = vy[idx]
    loss = np.float32(
        -np.log(np.mean(np.abs(hsic2) / (vx[:, None] * vs[None, :])) + EPS))
    return loss, mat, idx


def run_spmd(in_maps, trace=False):
    from concourse.bass_utils import run_bass_kernel_spmd
    return run_bass_kernel_spmd(_get_nc(), in_maps,
                                core_ids=list(range(N_CORES)), trace=trace)


def kernel(x, centroid_w):
    in_maps = _prepare_inputs(x, centroid_w)
    res = run_spmd(in_maps)
    return _postprocess(res.results)
